# revision 34
# baseline (speedup 1.0000x reference)
"""DilateAttention3D (3x3x3 window, dil=1) Trainium2 Bass kernel, 8-core SPMD.

Sharding: core = (b, dc) for b in {0,1}, dc in {0..3}: one batch element and a
D-chunk of 4 (halo 1 from zero-padded k/v) per core.

Per-core tile = (dz, y, xh): 6 heads x 16 queries = 96 voxels, key union
F = 3*3*18 = 162 positions (2 x'-half boxes of 81).

v6:
 - QK stationary reads a fully-unfolded resident k slab (one [128,81]
   contiguous slice per tile half) -- no per-tile gathers, no mask matmul:
   slab rows 96..112 hold an EXACT integer rank-17 factorization of the
   out-of-window mask (M = 1*1^T - I@band): qb row 96 = 1, rows 97+qi =
   one-hot(qi); slab row 96 = -BIG-SHIFT, rows 97+qi = +BIG*band[qi].
   SHIFT=20 is a global logit shift (cancelled by the host denominator
   divide) keeping exp() in fp16 range.
 - The k slab's unfolded k-rows (7.5MB if streamed) are built ON-CHIP by
   36 DVE copies from a compact x-duplicated slab (0.47MB); constant mask
   rows stream from DRAM. Build is chunked by dz and emitted one dz ahead
   of use so it pipelines with compute.
 - The block-diagonal q operand is built ON-CHIP by 6 Pool copies/group
   from a compact raw-q stream (0.79MB vs 4.7MB); off-diagonal zeros and
   mask rows live in 3 rotating persistent buffers, initialized once.
 - Everything fp16 (1 cyc/col on PE); fp32 PSUM accumulation.
 - DMA partition counts chosen so descriptors spread across all 16 DMA
   engines (96/128/48 -> 16-way, 33 -> 11-way; a prime like 113 would
   serialize on one engine).
 - exp batched 2 tiles per ACT instruction (two tiles share a PSUM bank);
   AV output pa [96(h,q), 97(h',c | denom)] written raw fp16 to DRAM;
   diagonal head-block extraction + 1/denom normalization on host.

Per tile: PE 2 QK + 2 AV matmuls | ACT 1/2 exp | DVE 1/2 copy.
"""
import os
import numpy as np

F16 = np.float16
B, d, D, H, W = 2, 96, 16, 32, 32
NH, HD = 6, 16
DL, DLH = 4, 6
NT = DL * H * 2        # 256 tiles/core
BIG = 200.0
SHIFT = 20.0
TB = 32
NG = NT // TB          # 8 groups
GPD = NG // DL         # groups per dz chunk (4)
R = 17
PC = 96 + R            # 113 contraction rows (padded to 128 on chip)

_cache = {}


def _build_nc():
    from concourse import bacc, mybir
    import concourse.tile as tile
    from contextlib import ExitStack

    f16 = mybir.dt.float16
    f32 = mybir.dt.float32
    nc = bacc.Bacc(None, target_bir_lowering=False, debug=True)

    qc_d = nc.declare_dram_parameter("qc", [NG, 96, TB, 96], f16, isOutput=False)
    maskq_d = nc.declare_dram_parameter("maskq", [32, TB, 96], f16, isOutput=False)
    ksx_d = nc.declare_dram_parameter("ksx", [96, DLH, 34, 2, 18], f16, isOutput=False)
    kmm_d = nc.declare_dram_parameter("kmm", [32, DL, H, 2, 2, 81], f16, isOutput=False)
    vt_d = nc.declare_dram_parameter("vt", [NG, 81, TB, 2, 97], f16, isOutput=False)
    out_d = nc.declare_dram_parameter("out", [NG, 96, TB, 97], f16, isOutput=True)

    with ExitStack() as ctx:
        tc = ctx.enter_context(tile.TileContext(nc))
        cpool = ctx.enter_context(tc.tile_pool(name="consts", bufs=1))
        qpool = ctx.enter_context(tc.tile_pool(name="q", bufs=3))
        vpool = ctx.enter_context(tc.tile_pool(name="vt", bufs=3))
        epool = ctx.enter_context(tc.tile_pool(name="es", bufs=4))
        opool = ctx.enter_context(tc.tile_pool(name="o", bufs=3))
        pspool = ctx.enter_context(tc.tile_pool(name="ps", bufs=4, space="PSUM"))
        papool = ctx.enter_context(tc.tile_pool(name="pa", bufs=4, space="PSUM"))

        # Compact k slab in two tiles (d 0..3 | 4..5) so the dz=0 unfold only
        # waits on the first, letting the PE start early.
        ksx_a = cpool.tile([96, 4, 34, 2, 18], f16, tag="ksxa", name="ksxa")
        ksx_b = cpool.tile([96, 2, 34, 2, 18], f16, tag="ksxb", name="ksxb")
        nc.sync.dma_start(ksx_a[:], ksx_d[:, 0:4])
        nc.sync.dma_start(ksx_b[:], ksx_d[:, 4:DLH])

        km_sbs = []
        for dz_ in range(DL):
            km_sbs.append(cpool.tile([128, H, 2, 2, 81], f16,
                                     tag=f"km{dz_}", name=f"km{dz_}"))

        def build_km(dz_):
            # rows 96..127: constant mask factor (+ zero pad), from DRAM
            nc.sync.dma_start(km_sbs[dz_][96:128], kmm_d[:, dz_])
            # rows 0..95: unfold k windows from the compact x2 slab
            for dzz in range(3):
                dd = dz_ + dzz
                src = ksx_a[:, dd] if dd < 4 else ksx_b[:, dd - 4]
                for yy in range(3):
                    s = 27 * dzz + 9 * yy
                    nc.vector.tensor_copy(
                        km_sbs[dz_][0:96, :, :, :, s:s + 9],
                        src[:, yy:yy + H, :, :],
                    )

        build_km(0)

        # 3 rotating q buffers [128, TB, 96]: rows 96..127 = constant mask
        # one-hots (DMA once), off-diagonal of rows 0..95 = 0 (memset once),
        # diagonal blocks written per group by Pool from the compact stream.
        qb_bufs = []
        for r_ in range(3):
            qbb = cpool.tile([128, TB, 96], f16, tag=f"qbuf{r_}", name=f"qbuf{r_}")
            qb_bufs.append(qbb)
            nc.sync.dma_start(qbb[96:128, :, :], maskq_d[:])

        for g_ in range(NG):
            qb = qb_bufs[g_ % 3]
            nc.sync.dma_start(qb[0:96, :, :], qc_d[g_])
            vt = vpool.tile([81, TB, 2, 97], f16, tag="vt")
            nc.gpsimd.dma_start(vt[0:48], vt_d[g_, 0:48])
            nc.gpsimd.dma_start(vt[48:81], vt_d[g_, 48:81])
            ob = opool.tile([96, TB, 97], f16, tag="ob")

            ps = None
            for i in range(TB):
                t = TB * g_ + i
                dz, rem = divmod(t, H * 2)
                y, xh = divmod(rem, 2)
                j = i % 2

                if j == 0:
                    ps = pspool.tile([81, 2, 2, 96], f32, tag="ps")
                for c in range(2):
                    nc.tensor.matmul(
                        ps[:, j, c, :],
                        lhsT=km_sbs[dz][:, y, xh, c, :],
                        rhs=qb[:, i, :], start=True, stop=True,
                    )
                if j == 1:
                    amt = epool.tile([81, 2, 2, 96], f16, tag="amt")
                    nc.scalar.activation(
                        amt[:], ps[:], mybir.ActivationFunctionType.Exp,
                        scale=0.25,
                    )
                    pa = papool.tile([96, 2, 97], f32, tag="pa")
                    for jj in range(2):
                        for c in range(2):
                            nc.tensor.matmul(
                                pa[:, jj, :],
                                lhsT=amt[:, jj, c, :],
                                rhs=vt[:, i - 1 + jj, c, :],
                                start=(c == 0), stop=(c == 1),
                            )
                    nc.vector.tensor_copy(ob[:, i - 1:i + 1, :], pa[:])
                if i == TB // 2 - 1:
                    nc.sync.dma_start(out_d[g_, :, 0:TB // 2],
                                      ob[:, 0:TB // 2, :])
            nc.sync.dma_start(out_d[g_, :, TB // 2:TB], ob[:, TB // 2:TB, :])
            # prefetch next dz chunk's k-slab build behind this group's loads
            if g_ % GPD == 0 and g_ // GPD + 1 < DL:
                build_km(g_ // GPD + 1)
    nc.compile()
    return nc


def _band():
    band = np.zeros((16, 18), np.float32)
    for qi in range(16):
        band[qi, qi:qi + 3] = 1.0
    return band


def _host_prep(q, k, v, b, dc):
    kp = np.pad(k[b], ((0, 0), (1, 1), (1, 1), (1, 1)))
    vp = np.pad(v[b], ((0, 0), (1, 1), (1, 1), (1, 1)))

    # compact x-duplicated k slab [96, 6, 34, 2, 18]
    ks = kp[:, 4 * dc:4 * dc + DLH]                       # [96,6,34,34]
    ksx = np.empty((96, DLH, 34, 2, 18), np.float32)
    ksx[..., 0, :] = ks[..., 0:18]
    ksx[..., 1, :] = ks[..., 16:34]

    # block-diagonal q stream [NG, 96, TB, 96] (rows 96..127 of the operand
    # are constant and live in the resident rotating buffers, see maskq).
    qr = q[b].reshape(NH, HD, D, H, W)[:, :, 4 * dc:4 * dc + DL]
    qrr = qr.reshape(NH, HD, NG, TB, 16)
    qc = np.zeros((NG, 96, TB, 96), np.float32)
    for h in range(NH):
        qc[:, 16 * h:16 * h + 16, :, 16 * h:16 * h + 16] = \
            qrr[h].transpose(1, 0, 2, 3)

    # vt [NG, 81, TB, 2, 97]: v windows (key order dz',y',xl) + ones col.
    v_slab = vp[:, 4 * dc:4 * dc + DLH]
    swv = np.lib.stride_tricks.sliding_window_view(
        v_slab, (3, 3, 18), axis=(1, 2, 3))
    wv = swv[:, :, :, ::16].transpose(1, 2, 3, 0, 4, 5, 6)      # [DL,H,2,96,3,3,18]
    vt = np.ones((DL, H, 2, 2, 81, 97), np.float32)
    wvt = wv.transpose(0, 1, 2, 4, 5, 6, 3)
    vt[..., 0, :, :96] = wvt[..., 0:9, :].reshape(DL, H, 2, 81, 96)
    vt[..., 1, :, :96] = wvt[..., 9:18, :].reshape(DL, H, 2, 81, 96)
    vt = vt.transpose(0, 1, 2, 4, 3, 5)                         # [DL,H,2,81,2,97]
    vt = vt.reshape(NG, TB, 81, 2, 97).transpose(0, 2, 1, 3, 4)  # [NG,81,TB,2,97]
    return (np.ascontiguousarray(qc.astype(F16)),
            np.ascontiguousarray(ksx.astype(F16)),
            np.ascontiguousarray(vt.astype(F16)))


def _consts():
    # maskq [32, TB, 96]: constant rows 96..127 of the q operand: row 0
    # (=96) all-ones, rows 1..16 (=97+qi) one-hot per query x-position.
    mq = np.zeros((32, TB, 96), np.float32)
    mq[0] = 1.0
    mq[1:17] = np.tile(np.eye(16, dtype=np.float32), (1, NH))[:, None, :]

    # kmm [32, DL, H, 2, 2, 81]: constant rows 96..127 of the k slab:
    # row 0 (=96) = -BIG-SHIFT, rows 1..16 (=97+qi) = +BIG*band.
    band = _band()
    bw = np.stack([band[:, 0:9], band[:, 9:18]], axis=1)  # [16,2,9]
    bw = np.broadcast_to(bw[:, None, None, None, :, None, None, :],
                         (16, DL, H, 2, 2, 3, 3, 9))
    kmm = np.zeros((32, DL, H, 2, 2, 81), np.float32)
    kmm[0] = -(BIG + SHIFT)
    kmm[1:17] = BIG * bw.reshape(16, DL, H, 2, 2, 81)
    return mq.astype(F16), kmm.astype(F16)


def kernel(q, k, v):
    q = np.asarray(q, np.float32)
    k = np.asarray(k, np.float32)
    v = np.asarray(v, np.float32)

    if "nc" not in _cache:
        _cache["nc"] = _build_nc()
    nc = _cache["nc"]

    from concourse.bass_utils import run_bass_kernel_spmd

    maskq, kmm = _consts()
    in_maps = []
    for core in range(8):
        b, dc = divmod(core, 4)
        qc, ksx, vt = _host_prep(q, k, v, b, dc)
        in_maps.append({"qc": qc, "ksx": ksx, "vt": vt,
                        "maskq": maskq, "kmm": kmm})

    res = run_bass_kernel_spmd(nc, in_maps, list(range(8)),
                               trace=bool(int(os.environ.get("KTRACE", "0"))))
    _cache["last_results"] = res

    hsel = np.arange(NH)
    full = np.zeros((B, D, H, W, d), np.float32)
    for core in range(8):
        b, dc = divmod(core, 4)
        ob = res.results[core]["out"].astype(np.float32)   # [NG, 96, TB, 97]
        pa = ob.transpose(0, 2, 1, 3).reshape(NT, 96, 97)
        den = pa[:, :, 96].reshape(NT, NH, 16)
        blocks = pa[:, :, :96].reshape(NT, NH, 16, NH, 16)
        o = blocks[:, hsel, :, hsel, :]                    # [NH, NT, 16, 16]
        o = o.transpose(1, 0, 2, 3) / den[:, :, :, None]   # [NT, NH, 16q, 16c]
        o = o.reshape(DL, H, 2, NH, 16, 16).transpose(0, 1, 2, 4, 3, 5)
        full[b, 4 * dc:4 * dc + DL] = o.reshape(DL, H, W, d)
    return full
